# revision 1
# baseline (speedup 1.0000x reference)
"""GNN message-passing layer (normalized-adjacency conv + linear + LeakyReLU)
on 8 Trainium2 NeuronCores, pure data parallel over the batch dim.

Computation (per batch b):
    deg      = adj.sum(-1)                     # [N]
    agg      = (adj / deg[:, None]) @ X        # [N, FIN]
    out      = leakyrelu(agg @ W.T + bias)     # [N, FOUT]

Device-side formulation. adj is host-transposed per batch (adjT[k, m] =
adj[m, k]) so the contraction index k sits on SBUF partitions for both matmul
operands, and everything downstream stays transposed ([feature, node] order)
so all PE work streams 512-wide:
    rawT[f, m]   = sum_k X[k, f] * adjT[k, m]    # X tiles as weights, fp32r
    degbc[:, m]  = sum_k 1 * adjT[k, m]          # ones[128,128] weights ->
                                                 # deg broadcast to all parts
    out2T[o, m]  = sum_f WT[f, o] * rawT[f, m]   # W as weights, fp32r
    t            = out2T / degbc                 # DVE divide
    outT[o, m]   = alpha*(t + b) + (1-alpha)*Relu(t + b)   # b is per-partition
The DRAM output is [B, FOUT, N]; the host swaps the last two axes.

The matmuls run in fp32r (fp32 with 11 explicit mantissa bits; 1 PE cycle/row
instead of 4): adjT/x/wT are pre-rounded to fp32r on the host
(round-to-nearest-even on the dropped 12 bits) and declared float32r
end-to-end; rawT is rounded to fp32r by the PSUM->SBUF copy. deg multiplies
the rounded values by exactly-representable 1.0, so deg is exact w.r.t. the
rounded adjacency; bias stays exact fp32.
"""

import numpy as np

import concourse.bass as bass
import concourse.mybir as mybir
import concourse.tile as tile
from concourse.bass_utils import run_bass_kernel_spmd

P = 128

# Problem shape (hardcoded per the harness contract).
B, N, FIN, FOUT = 32, 1024, 128, 128
NEG_SLOPE = 0.01
N_CORES = 8
BPC = B // N_CORES  # batches per core


def build_bass(nbatch=BPC, n=N, fin=FIN, fout=FOUT, neg_slope=NEG_SLOPE,
               adj_bufs=5, use_f32r=True, f32r_second=True):
    f32 = mybir.dt.float32
    mmdt = mybir.dt.float32r if use_f32r else f32
    rdt = mybir.dt.float32r if (use_f32r and f32r_second) else f32
    alpha = float(neg_slope)
    nc = bass.Bass()

    adjT = nc.dram_tensor("adjT", [nbatch, n, n], mmdt, kind="ExternalInput")
    x = nc.dram_tensor("x", [nbatch, P, n // P, fin], mmdt,
                       kind="ExternalInput")
    onesW = nc.dram_tensor("onesW", [P, P], mmdt, kind="ExternalInput")
    wT = nc.dram_tensor("wT", [fin, fout], rdt, kind="ExternalInput")
    bvec = nc.dram_tensor("bvec", [P, 1], f32, kind="ExternalInput")
    outT = nc.dram_tensor("outT", [nbatch, fout, n], f32, kind="ExternalOutput")

    KT = n // P          # contraction tiles
    CH = min(512, n)     # matmul moving free dim (one fp32 PSUM bank)
    NCH = n // CH        # moving-dim chunks

    with tile.TileContext(nc) as tc:
        with (
            tc.tile_pool(name="const", bufs=1) as cpool,
            tc.tile_pool(name="adj", bufs=adj_bufs) as apool,
            tc.tile_pool(name="xt", bufs=2) as xpool,
            tc.tile_pool(name="raw", bufs=2) as rpool,
            tc.tile_pool(name="post", bufs=4) as opool,
            tc.tile_pool(name="psr", bufs=3, space="PSUM") as ps_raw,
            tc.tile_pool(name="psd", bufs=2, space="PSUM") as ps_deg,
            tc.tile_pool(name="pso", bufs=2, space="PSUM") as ps_out,
        ):
            wT_sb = cpool.tile([fin, fout], rdt, tag="w")
            nc.sync.dma_start(wT_sb[:], wT[:, :])
            b_sb = cpool.tile([P, 1], f32, tag="b")
            nc.sync.dma_start(b_sb[:], bvec[:, :])
            # (1-alpha)*b for the fused Relu bias
            b2_sb = cpool.tile([P, 1], f32, tag="b2")
            nc.vector.tensor_scalar_mul(b2_sb[:], b_sb[:], 1.0 - alpha)
            onesW_sb = cpool.tile([P, P], mmdt, tag="onesW")
            nc.sync.dma_start(onesW_sb[:], onesW[:, :])

            for b in range(nbatch):
                x_sb = xpool.tile([P, KT, fin], mmdt, tag="x")
                nc.sync.dma_start(x_sb[:], x[b])

                # adj in two 2 MB dma_starts (>=1 MiB per transfer for full
                # SDMA fan-out), each carrying KG k-tiles
                KG = KT // 2
                adj_chunks = []
                for c2 in range(2):
                    ac = apool.tile([P, KG, n], mmdt, tag="adj", name=f"ac{c2}")
                    nc.sync.dma_start(
                        ac[:],
                        adjT[b, c2 * KG * P:(c2 + 1) * KG * P, :]
                        .rearrange("(g p) m -> p g m", p=P),
                    )
                    adj_chunks.append(ac)

                def adj_slice(k, c):
                    return adj_chunks[k // KG][:, k % KG, c * CH:(c + 1) * CH]

                # rawT matmuls, one accumulation group per 512-chunk
                ps_chunks = [
                    ps_raw.tile([P, CH], f32, tag="psraw", name=f"psraw{cc}")
                    for cc in range(NCH)
                ]
                for k in range(KT):
                    for c in range(NCH):
                        nc.tensor.matmul(
                            ps_chunks[c][:, :],
                            x_sb[:, k, :],
                            adj_slice(k, c),
                            start=(k == 0),
                            stop=(k == KT - 1),
                        )

                # Partial k-tile sums for deg on the DVE (tree, 7 adds);
                # the ones-weights matmul below folds the remaining 128
                # partitions and broadcasts deg to every output partition.
                def aslc(k):
                    return adj_chunks[k // KG][:, k % KG, :]

                half = KT // 2
                acc_a = rpool.tile([P, n], mmdt, tag="acca")
                nc.vector.tensor_tensor(
                    acc_a[:, :], aslc(0), aslc(1), mybir.AluOpType.add)
                for k in range(2, half):
                    nc.vector.tensor_tensor(
                        acc_a[:, :], acc_a[:, :], aslc(k), mybir.AluOpType.add)
                acc = rpool.tile([P, n], mmdt, tag="accc")
                if KT > 2:
                    acc_b = rpool.tile([P, n], mmdt, tag="accb")
                    nc.vector.tensor_tensor(
                        acc_b[:, :], aslc(half), aslc(half + 1),
                        mybir.AluOpType.add)
                    for k in range(half + 2, KT):
                        nc.vector.tensor_tensor(
                            acc_b[:, :], acc_b[:, :], aslc(k),
                            mybir.AluOpType.add)
                    nc.vector.tensor_tensor(
                        acc[:, :], acc_a[:, :], acc_b[:, :], mybir.AluOpType.add)
                else:
                    nc.vector.tensor_copy(acc[:, :], acc_a[:, :])

                raw_sb = rpool.tile([P, n], rdt, tag="raw")
                for c in range(NCH):
                    nc.scalar.copy(raw_sb[:, c * CH:(c + 1) * CH], ps_chunks[c][:, :])

                o_full = opool.tile([P, n], f32, tag="ofull")
                for c in range(NCH):
                    # deg broadcast to all partitions via ones weights
                    ps_db = ps_deg.tile([P, CH], f32, tag="psdeg")
                    nc.tensor.matmul(
                        ps_db[:, :],
                        onesW_sb[:, :],
                        acc[:, c * CH:(c + 1) * CH],
                        start=True,
                        stop=True,
                    )
                    # 1/deg on the scalar engine (reciprocal LUT; its error is
                    # quadratically suppressed nowhere here, so the HW rel-err
                    # check guards it). bass refuses Reciprocal directly, so
                    # emit a Copy and flip the func.
                    rec_sb = opool.tile([P, CH], f32, tag="rec")
                    _ai = nc.scalar.activation(
                        rec_sb[:, :], ps_db[:, :],
                        mybir.ActivationFunctionType.Copy, bias=0.0, scale=1.0)
                    _ai.ins.func = mybir.ActivationFunctionType.Reciprocal

                    # out2T[o, m] = sum_f WT[f, o] * rawT[f, m]
                    ps_o = ps_out.tile([P, CH], f32, tag="psout")
                    nc.tensor.matmul(
                        ps_o[:, :],
                        wT_sb[:, :],
                        raw_sb[:, c * CH:(c + 1) * CH],
                        start=True,
                        stop=True,
                    )
                    # t = out2T / deg
                    t_sb = opool.tile([P, CH], f32, tag="t")
                    nc.vector.tensor_tensor(
                        t_sb[:, :], ps_o[:, :], rec_sb[:, :],
                        mybir.AluOpType.mult,
                    )
                    # u = alpha * (t + b)
                    u_sb = opool.tile([P, CH], f32, tag="u")
                    nc.vector.tensor_scalar(
                        u_sb[:, :], t_sb[:, :], b_sb[:, 0:1], alpha,
                        mybir.AluOpType.add, mybir.AluOpType.mult,
                    )
                    # r = Relu((1-alpha)*t + (1-alpha)*b) = (1-alpha)*Relu(t+b)
                    r_sb = opool.tile([P, CH], f32, tag="r")
                    nc.scalar.activation(
                        r_sb[:, :], t_sb[:, :],
                        mybir.ActivationFunctionType.Relu,
                        bias=b2_sb[:, 0:1], scale=1.0 - alpha,
                    )
                    # outT = u + r = leaky(t + b)
                    nc.vector.tensor_tensor(
                        o_full[:, c * CH:(c + 1) * CH], u_sb[:, :], r_sb[:, :],
                        mybir.AluOpType.add,
                    )
                nc.sync.dma_start(outT[b], o_full[:, :])

    _split_multi_waits(nc)
    return nc


def _split_multi_waits(nc):
    """Walrus rejects split-struct instructions (fp32/fp32r fused-weight-load
    matmult, TensorScalarPtr, ...) with more than one sync wait ("Too many
    sync wait commands" in setupSyncWait<...>). Hoist all but the last wait
    of each multi-wait instruction onto same-engine no-ops inserted
    immediately before it (one wait per no-op)."""
    cnt = 0
    for f in nc.m.functions:
        for blk in f.blocks:
            idx = 0
            while idx < len(blk.instructions):
                inst = blk.instructions[idx]
                si = inst.sync_info
                if (type(inst).__name__ != "InstNoOp" and si is not None
                        and len(si.on_wait) > 1):
                    waits = list(si.on_wait)
                    for w in waits[:-1]:
                        nop = mybir.InstNoOp(name=f"mm_wait_nop_{cnt}",
                                             ins=[], outs=[])
                        cnt += 1
                        nop.engine = inst.engine
                        nop.sync_info = mybir.SyncInfo(on_wait=[w],
                                                       on_update=[])
                        nc.register_instruction(nop)
                        blk.instructions.insert(idx, nop)
                        idx += 1
                    inst.sync_info = mybir.SyncInfo(
                        on_wait=waits[-1:], on_update=list(si.on_update))
                idx += 1
    return cnt


_NC_CACHE = {}

USE_F32R = True
F32R_SECOND = True


def _get_nc():
    if "nc" not in _NC_CACHE:
        _NC_CACHE["nc"] = build_bass(use_f32r=USE_F32R, f32r_second=F32R_SECOND)
    return _NC_CACHE["nc"]


def _round_fp32r(a):
    """Round fp32 values to fp32r (11 explicit mantissa bits), RNE."""
    u = np.ascontiguousarray(a, dtype=np.float32).view(np.uint32)
    r = (u + np.uint32(0x7FF) + ((u >> np.uint32(12)) & np.uint32(1))) \
        & np.uint32(0xFFFFF000)
    return r.view(np.float32)


def _prep_in_maps(node_mat, adj_mat, W, b):
    node_mat = np.ascontiguousarray(node_mat, dtype=np.float32)
    adj_mat = np.asarray(adj_mat, dtype=np.float32)
    wT = np.ascontiguousarray(np.asarray(W, dtype=np.float32).T)
    if USE_F32R and F32R_SECOND:
        wT = _round_fp32r(wT)
    bvec = np.ascontiguousarray(
        np.asarray(b, dtype=np.float32).reshape(P, 1))
    onesW = np.ones((P, P), dtype=np.float32)
    in_maps = []
    for c in range(N_CORES):
        sl = slice(c * BPC, (c + 1) * BPC)
        adjT = np.ascontiguousarray(adj_mat[sl].transpose(0, 2, 1))
        xs = np.ascontiguousarray(
            node_mat[sl].reshape(BPC, N // P, P, FIN).transpose(0, 2, 1, 3))
        if USE_F32R:
            adjT = _round_fp32r(adjT)
            xs = _round_fp32r(xs)
        in_maps.append({
            "adjT": adjT,
            "x": xs,
            "onesW": onesW,
            "wT": wT,
            "bvec": bvec,
        })
    return in_maps


def kernel(node_mat, adj_mat, W, b):
    nc = _get_nc()
    in_maps = _prep_in_maps(node_mat, adj_mat, W, b)
    res = run_bass_kernel_spmd(nc, in_maps, core_ids=list(range(N_CORES)))
    return np.ascontiguousarray(
        np.concatenate(
            [res.results[c]["outT"] for c in range(N_CORES)], axis=0
        ).swapaxes(1, 2)
    )



# revision 2
# speedup vs baseline: 1.4971x; 1.4971x over previous
"""GNN message-passing layer (normalized-adjacency conv + linear + LeakyReLU)
on 8 Trainium2 NeuronCores, pure data parallel over the batch dim.

Computation (per batch b):
    deg      = adj.sum(-1)                     # [N]
    agg      = (adj / deg[:, None]) @ X        # [N, FIN]
    out      = leakyrelu(agg @ W.T + bias)     # [N, FOUT]

Device-side formulation. adj is host-transposed per batch (adjT[k, m] =
adj[m, k]) so the contraction index k sits on SBUF partitions for both matmul
operands, and everything downstream stays transposed ([feature, node] order)
so all PE work streams 512-wide:
    rawT[f, m]   = sum_k X[k, f] * adjT[k, m]    # X tiles as weights
    degbc[:, m]  = sum_k 1 * adjT[k, m]          # ones weights -> deg
                                                 # broadcast to all partitions
    out2T[o, m]  = sum_f WT[f, o] * rawT[f, m]   # W as weights
    t            = out2T * (1/degbc)             # DVE multiply
    outT[o, m]   = Lrelu(t + b)                  # scalar engine, per-partition b
The DRAM output is [B, FOUT, N] fp16; the host swaps the last two axes and
casts to fp32.

Everything DMA'd is fp16 (half the HBM traffic of fp32; adj/X values are
well inside fp16 range and the 2^-11 rounding is far below the accuracy
gate). The deg reduction over the 8 k-tiles is split: 4 pairwise adds on the
DVE (fp16, 2x mode) fold 8 tiles to 4, then a 4-matmul PSUM accumulation
with ones weights folds the rest and broadcasts deg to all 128 partitions.
"""

import numpy as np

import concourse.bass as bass
import concourse.mybir as mybir
import concourse.tile as tile
from concourse.bass_utils import run_bass_kernel_spmd

P = 128

# Problem shape (hardcoded per the harness contract).
B, N, FIN, FOUT = 32, 1024, 128, 128
NEG_SLOPE = 0.01
N_CORES = 8
BPC = B // N_CORES  # batches per core

USE_LRELU = True


def build_bass(nbatch=BPC, n=N, fin=FIN, fout=FOUT, neg_slope=NEG_SLOPE,
               adj_bufs=6, use_lrelu=USE_LRELU):
    f32 = mybir.dt.float32
    f16 = mybir.dt.float16
    alpha = float(neg_slope)
    nc = bass.Bass()

    adjT = nc.dram_tensor("adjT", [nbatch, n, n], f16, kind="ExternalInput")
    x = nc.dram_tensor("x", [nbatch, P, n // P, fin], f16,
                       kind="ExternalInput")
    onesW = nc.dram_tensor("onesW", [P, P], f16, kind="ExternalInput")
    wT = nc.dram_tensor("wT", [fin, fout], f16, kind="ExternalInput")
    bvec = nc.dram_tensor("bvec", [P, 1], f32, kind="ExternalInput")
    outT = nc.dram_tensor("outT", [nbatch, fout, n], f16, kind="ExternalOutput")

    KT = n // P          # contraction tiles (8)
    CH = min(512, n)     # matmul moving free dim (one fp32 PSUM bank)
    NCH = n // CH        # moving-dim chunks (2)
    KG = KT // 2         # k-tiles per adj DMA chunk (4)

    with tile.TileContext(nc) as tc:
        with (
            tc.tile_pool(name="const", bufs=1) as cpool,
            tc.tile_pool(name="adj", bufs=adj_bufs) as apool,
            tc.tile_pool(name="xt", bufs=1) as xpool,
            tc.tile_pool(name="raw", bufs=2) as rpool,
            tc.tile_pool(name="post", bufs=4) as opool,
            tc.tile_pool(name="psr", bufs=3, space="PSUM") as ps_raw,
            tc.tile_pool(name="psd", bufs=2, space="PSUM") as ps_deg,
            tc.tile_pool(name="pso", bufs=2, space="PSUM") as ps_out,
        ):
            wT_sb = cpool.tile([fin, fout], f16, tag="w")
            nc.sync.dma_start(wT_sb[:], wT[:, :])
            b_sb = cpool.tile([P, 1], f32, tag="b")
            nc.sync.dma_start(b_sb[:], bvec[:, :])
            if not use_lrelu:
                b2_sb = cpool.tile([P, 1], f32, tag="b2")
                nc.vector.tensor_scalar_mul(b2_sb[:], b_sb[:], 1.0 - alpha)
            onesW_sb = cpool.tile([P, P], f16, tag="onesW")
            nc.sync.dma_start(onesW_sb[:], onesW[:, :])
            # all 4 batches of X in one 1 MiB DMA
            x_sb = xpool.tile([P, nbatch, KT, fin], f16, tag="x")
            nc.sync.dma_start(x_sb[:], x.rearrange("b p k f -> p b k f"))

            for b in range(nbatch):
                # adj in two 1 MiB dma_starts, each carrying KG k-tiles
                adj_chunks = []
                for c2 in range(2):
                    ac = apool.tile([P, KG, n], f16, tag="adj", name=f"ac{c2}")
                    nc.sync.dma_start(
                        ac[:],
                        adjT[b, c2 * KG * P:(c2 + 1) * KG * P, :]
                        .rearrange("(g p) m -> p g m", p=P),
                    )
                    adj_chunks.append(ac)

                def adj_slice(k, c):
                    return adj_chunks[k // KG][:, k % KG, c * CH:(c + 1) * CH]

                # rawT matmuls, one accumulation group per 512-chunk
                ps_chunks = [
                    ps_raw.tile([P, CH], f32, tag="psraw", name=f"psraw{cc}")
                    for cc in range(NCH)
                ]
                for k in range(KT):
                    for c in range(NCH):
                        nc.tensor.matmul(
                            ps_chunks[c][:, :],
                            x_sb[:, b, k, :],
                            adj_slice(k, c),
                            start=(k == 0),
                            stop=(k == KT - 1),
                        )

                # deg: fold 8 k-tiles to 4 with pairwise DVE adds (fp16 2x);
                # the accumulating ones-weights matmuls below fold the rest
                # and broadcast deg to every output partition.
                def aslc(k):
                    return adj_chunks[k // KG][:, k % KG, :]

                pa = []
                for g in range(KT // 2):
                    pt = rpool.tile([P, n], f16, tag=f"pa{g}")
                    nc.vector.tensor_tensor(
                        pt[:, :], aslc(2 * g), aslc(2 * g + 1),
                        mybir.AluOpType.add)
                    pa.append(pt)

                raw_sb = rpool.tile([P, n], f16, tag="raw")
                for c in range(NCH):
                    nc.scalar.copy(raw_sb[:, c * CH:(c + 1) * CH],
                                   ps_chunks[c][:, :])

                o_full = opool.tile([P, n], f16, tag="ofull")
                for c in range(NCH):
                    # deg broadcast to all partitions via ones weights
                    ps_db = ps_deg.tile([P, CH], f32, tag="psdeg")
                    for g in range(KT // 2):
                        nc.tensor.matmul(
                            ps_db[:, :],
                            onesW_sb[:, :],
                            pa[g][:, c * CH:(c + 1) * CH],
                            start=(g == 0),
                            stop=(g == KT // 2 - 1),
                        )
                    # 1/deg on the scalar engine (reciprocal LUT). bass
                    # refuses Reciprocal directly, so emit a Copy and flip
                    # the func.
                    rec_sb = opool.tile([P, CH], f32, tag="rec")
                    _ai = nc.scalar.activation(
                        rec_sb[:, :], ps_db[:, :],
                        mybir.ActivationFunctionType.Copy, bias=0.0, scale=1.0)
                    _ai.ins.func = mybir.ActivationFunctionType.Reciprocal

                    # out2T[o, m] = sum_f WT[f, o] * rawT[f, m]
                    ps_o = ps_out.tile([P, CH], f32, tag="psout")
                    nc.tensor.matmul(
                        ps_o[:, :],
                        wT_sb[:, :],
                        raw_sb[:, c * CH:(c + 1) * CH],
                        start=True,
                        stop=True,
                    )
                    # t = out2T / deg
                    t_sb = opool.tile([P, CH], f32, tag="t")
                    nc.vector.tensor_tensor(
                        t_sb[:, :], ps_o[:, :], rec_sb[:, :],
                        mybir.AluOpType.mult,
                    )
                    if use_lrelu:
                        # outT = Lrelu(t + b), negative slope alpha
                        nc.scalar.activation(
                            o_full[:, c * CH:(c + 1) * CH], t_sb[:, :],
                            mybir.ActivationFunctionType.Lrelu,
                            bias=b_sb[:, 0:1], scale=1.0, alpha=alpha,
                        )
                    else:
                        # u = alpha * (t + b)
                        u_sb = opool.tile([P, CH], f32, tag="u")
                        nc.vector.tensor_scalar(
                            u_sb[:, :], t_sb[:, :], b_sb[:, 0:1], alpha,
                            mybir.AluOpType.add, mybir.AluOpType.mult,
                        )
                        # r = Relu((1-a)*t + (1-a)*b) = (1-a)*Relu(t+b)
                        r_sb = opool.tile([P, CH], f32, tag="r")
                        nc.scalar.activation(
                            r_sb[:, :], t_sb[:, :],
                            mybir.ActivationFunctionType.Relu,
                            bias=b2_sb[:, 0:1], scale=1.0 - alpha,
                        )
                        # outT = u + r = leaky(t + b)
                        nc.vector.tensor_tensor(
                            o_full[:, c * CH:(c + 1) * CH], u_sb[:, :],
                            r_sb[:, :], mybir.AluOpType.add,
                        )
                nc.sync.dma_start(outT[b], o_full[:, :])

    _split_multi_waits(nc)
    return nc


def _split_multi_waits(nc):
    """Walrus rejects split-struct instructions with more than one sync wait
    ("Too many sync wait commands" in setupSyncWait<...>). Hoist all but the
    last wait of each multi-wait instruction onto same-engine no-ops inserted
    immediately before it (one wait per no-op)."""
    cnt = 0
    for f in nc.m.functions:
        for blk in f.blocks:
            idx = 0
            while idx < len(blk.instructions):
                inst = blk.instructions[idx]
                si = inst.sync_info
                if (type(inst).__name__ != "InstNoOp" and si is not None
                        and len(si.on_wait) > 1):
                    waits = list(si.on_wait)
                    for w in waits[:-1]:
                        nop = mybir.InstNoOp(name=f"mm_wait_nop_{cnt}",
                                             ins=[], outs=[])
                        cnt += 1
                        nop.engine = inst.engine
                        nop.sync_info = mybir.SyncInfo(on_wait=[w],
                                                       on_update=[])
                        nc.register_instruction(nop)
                        blk.instructions.insert(idx, nop)
                        idx += 1
                    inst.sync_info = mybir.SyncInfo(
                        on_wait=waits[-1:], on_update=list(si.on_update))
                idx += 1
    return cnt


_NC_CACHE = {}


def _get_nc():
    if "nc" not in _NC_CACHE:
        _NC_CACHE["nc"] = build_bass()
    return _NC_CACHE["nc"]


def _prep_in_maps(node_mat, adj_mat, W, b):
    node_mat = np.asarray(node_mat, dtype=np.float32)
    adj_mat = np.asarray(adj_mat, dtype=np.float32)
    wT = np.ascontiguousarray(np.asarray(W, dtype=np.float32).T
                              .astype(np.float16))
    bvec = np.ascontiguousarray(
        np.asarray(b, dtype=np.float32).reshape(P, 1))
    onesW = np.ones((P, P), dtype=np.float16)
    in_maps = []
    for c in range(N_CORES):
        sl = slice(c * BPC, (c + 1) * BPC)
        adjT = np.ascontiguousarray(
            adj_mat[sl].astype(np.float16).transpose(0, 2, 1))
        xs = np.ascontiguousarray(
            node_mat[sl].astype(np.float16)
            .reshape(BPC, N // P, P, FIN).transpose(0, 2, 1, 3))
        in_maps.append({
            "adjT": adjT,
            "x": xs,
            "onesW": onesW,
            "wT": wT,
            "bvec": bvec,
        })
    return in_maps


def kernel(node_mat, adj_mat, W, b):
    nc = _get_nc()
    in_maps = _prep_in_maps(node_mat, adj_mat, W, b)
    res = run_bass_kernel_spmd(nc, in_maps, core_ids=list(range(N_CORES)))
    return np.ascontiguousarray(
        np.concatenate(
            [res.results[c]["outT"] for c in range(N_CORES)], axis=0
        ).swapaxes(1, 2).astype(np.float32)
    )


# revision 5
# speedup vs baseline: 1.5717x; 1.0499x over previous
"""GNN message-passing layer (normalized-adjacency conv + linear + LeakyReLU)
on 8 Trainium2 NeuronCores, pure data parallel over the batch dim.

Computation (per batch b):
    deg      = adj.sum(-1)                     # [N]
    agg      = (adj / deg[:, None]) @ X        # [N, FIN]
    out      = leakyrelu(agg @ W.T + bias)     # [N, FOUT]

Device-side formulation. adj is host-transposed per batch (adjT[k, m] =
adj[m, k]) so the contraction index k sits on SBUF partitions for both matmul
operands, and everything downstream stays transposed ([feature, node] order)
so all PE work streams 512-wide:
    rawT[f, m]   = sum_k X[k, f] * adjT[k, m]    # X tiles as weights
    degbc[:, m]  = sum_k 1 * adjT[k, m]          # ones weights -> deg
                                                 # broadcast to all partitions
    out2T[o, m]  = sum_f WT[f, o] * rawT[f, m]   # W as weights
    t            = out2T * (1/degbc)             # DVE multiply
    outT[o, m]   = Lrelu(t + b)                  # scalar engine, per-partition b
The DRAM output is [B, FOUT, N] fp16; the host swaps the last two axes and
casts to fp32.

Everything DMA'd is fp16 (half the HBM traffic of fp32; adj/X values are
well inside fp16 range and the 2^-11 rounding is far below the accuracy
gate). The deg reduction over the 8 k-tiles is split: 4 pairwise adds on the
DVE (fp16, 2x mode) fold 8 tiles to 4, then a 4-matmul PSUM accumulation
with ones weights folds the rest and broadcasts deg to all 128 partitions.
"""

import numpy as np

import concourse.bass as bass
import concourse.mybir as mybir
import concourse.tile as tile
from concourse.bass_utils import run_bass_kernel_spmd

P = 128

# Problem shape (hardcoded per the harness contract).
B, N, FIN, FOUT = 32, 1024, 128, 128
NEG_SLOPE = 0.01
N_CORES = 8
BPC = B // N_CORES  # batches per core

USE_LRELU = False


def build_bass(nbatch=BPC, n=N, fin=FIN, fout=FOUT, neg_slope=NEG_SLOPE,
               adj_bufs=8, use_lrelu=USE_LRELU):
    f32 = mybir.dt.float32
    f16 = mybir.dt.float16
    alpha = float(neg_slope)
    nc = bass.Bass()

    adjT = nc.dram_tensor("adjT", [nbatch, n, n], f16, kind="ExternalInput")
    x = nc.dram_tensor("x", [nbatch, P, n // P, fin], f16,
                       kind="ExternalInput")
    onesW = nc.dram_tensor("onesW", [P, P], f16, kind="ExternalInput")
    wT = nc.dram_tensor("wT", [fin, fout], f16, kind="ExternalInput")
    bvec = nc.dram_tensor("bvec", [P, 1], f32, kind="ExternalInput")
    outT = nc.dram_tensor("outT", [nbatch, fout, n], f16, kind="ExternalOutput")

    KT = n // P          # contraction tiles (8)
    CH = min(512, n)     # matmul moving free dim (one fp32 PSUM bank)
    NCH = n // CH        # moving-dim chunks (2)
    KG = KT // 2         # k-tiles per adj DMA chunk (4)

    with tile.TileContext(nc) as tc:
        with (
            tc.tile_pool(name="const", bufs=1) as cpool,
            tc.tile_pool(name="adj", bufs=adj_bufs) as apool,
            tc.tile_pool(name="xt", bufs=1) as xpool,
            tc.tile_pool(name="raw", bufs=2) as rpool,
            tc.tile_pool(name="post", bufs=4) as opool,
            tc.tile_pool(name="psr", bufs=3, space="PSUM") as ps_raw,
            tc.tile_pool(name="psd", bufs=2, space="PSUM") as ps_deg,
            tc.tile_pool(name="pso", bufs=2, space="PSUM") as ps_out,
        ):
            wT_sb = cpool.tile([fin, fout], f16, tag="w")
            nc.sync.dma_start(wT_sb[:], wT[:, :])
            b_sb = cpool.tile([P, 1], f32, tag="b")
            nc.sync.dma_start(b_sb[:], bvec[:, :])
            if not use_lrelu:
                b2_sb = cpool.tile([P, 1], f32, tag="b2")
                nc.vector.tensor_scalar_mul(b2_sb[:], b_sb[:], 1.0 - alpha)
            onesW_sb = cpool.tile([P, P], f16, tag="onesW")
            nc.sync.dma_start(onesW_sb[:], onesW[:, :])
            # all 4 batches of X in one 1 MiB DMA
            x_sb = xpool.tile([P, nbatch, KT, fin], f16, tag="x")
            nc.sync.dma_start(x_sb[:], x.rearrange("b p k f -> p b k f"))

            # Prefetch ALL adjacency DMAs up-front (8 x 1 MiB): the DMA
            # engines then stream continuously instead of waiting on the
            # per-batch dependency chain. adj_bufs covers every chunk.
            all_chunks = []
            for b in range(nbatch):
                for c2 in range(2):
                    ac = apool.tile([P, KG, n], f16, tag="adj",
                                    name=f"ac{b}_{c2}")
                    nc.sync.dma_start(
                        ac[:],
                        adjT[b, c2 * KG * P:(c2 + 1) * KG * P, :]
                        .rearrange("(g p) m -> p g m", p=P),
                    )
                    all_chunks.append(ac)

            for b in range(nbatch):
                adj_chunks = all_chunks[2 * b:2 * b + 2]

                def adj_slice(k, c):
                    return adj_chunks[k // KG][:, k % KG, c * CH:(c + 1) * CH]

                # rawT matmuls, one accumulation group per 512-chunk
                ps_chunks = [
                    ps_raw.tile([P, CH], f32, tag="psraw", name=f"psraw{cc}")
                    for cc in range(NCH)
                ]
                for k in range(KT):
                    for c in range(NCH):
                        nc.tensor.matmul(
                            ps_chunks[c][:, :],
                            x_sb[:, b, k, :],
                            adj_slice(k, c),
                            start=(k == 0),
                            stop=(k == KT - 1),
                        )

                # deg: fold 8 k-tiles to 4 with pairwise DVE adds (fp16 2x);
                # the accumulating ones-weights matmuls below fold the rest
                # and broadcast deg to every output partition.
                def aslc(k):
                    return adj_chunks[k // KG][:, k % KG, :]

                pa = []
                for g in range(KT // 2):
                    pt = rpool.tile([P, n], f16, tag=f"pa{g}")
                    nc.vector.tensor_tensor(
                        pt[:, :], aslc(2 * g), aslc(2 * g + 1),
                        mybir.AluOpType.add)
                    pa.append(pt)

                raw_sb = rpool.tile([P, n], f16, tag="raw")
                for c in range(NCH):
                    nc.scalar.copy(raw_sb[:, c * CH:(c + 1) * CH],
                                   ps_chunks[c][:, :])

                o_full = opool.tile([P, n], f16, tag="ofull")
                for c in range(NCH):
                    # deg broadcast to all partitions via ones weights
                    ps_db = ps_deg.tile([P, CH], f32, tag="psdeg")
                    for g in range(KT // 2):
                        nc.tensor.matmul(
                            ps_db[:, :],
                            onesW_sb[:, :],
                            pa[g][:, c * CH:(c + 1) * CH],
                            start=(g == 0),
                            stop=(g == KT // 2 - 1),
                        )
                    # 1/deg on the scalar engine (reciprocal LUT). bass
                    # refuses Reciprocal directly, so emit a Copy and flip
                    # the func.
                    rec_sb = opool.tile([P, CH], f32, tag="rec")
                    _ai = nc.scalar.activation(
                        rec_sb[:, :], ps_db[:, :],
                        mybir.ActivationFunctionType.Copy, bias=0.0, scale=1.0)
                    _ai.ins.func = mybir.ActivationFunctionType.Reciprocal

                    # out2T[o, m] = sum_f WT[f, o] * rawT[f, m]
                    ps_o = ps_out.tile([P, CH], f32, tag="psout")
                    nc.tensor.matmul(
                        ps_o[:, :],
                        wT_sb[:, :],
                        raw_sb[:, c * CH:(c + 1) * CH],
                        start=True,
                        stop=True,
                    )
                    # t = out2T / deg (fp16 out: faster 16-bit DVE modes
                    # downstream; ~2^-11 relative rounding, negligible)
                    t_sb = opool.tile([P, CH], f16, tag="t")
                    nc.vector.tensor_tensor(
                        t_sb[:, :], ps_o[:, :], rec_sb[:, :],
                        mybir.AluOpType.mult,
                    )
                    if use_lrelu:
                        # outT = Lrelu(t + b), negative slope alpha.
                        # (Unused by default: Lrelu lives in a different
                        # ACT table set than Reciprocal, and the per-batch
                        # ACT_TABLE_LOAD thrash costs ~10 us/core.)
                        nc.scalar.activation(
                            o_full[:, c * CH:(c + 1) * CH], t_sb[:, :],
                            mybir.ActivationFunctionType.Lrelu,
                            bias=b_sb[:, 0:1], scale=1.0, alpha=alpha,
                        )
                    else:
                        # u = alpha * (t + b)
                        u_sb = opool.tile([P, CH], f16, tag="u")
                        nc.vector.tensor_scalar(
                            u_sb[:, :], t_sb[:, :], b_sb[:, 0:1], alpha,
                            mybir.AluOpType.add, mybir.AluOpType.mult,
                        )
                        # r = Relu((1-a)*t + (1-a)*b) = (1-a)*Relu(t+b);
                        # Relu is a filler function present in every ACT
                        # table set, so no set switch vs Reciprocal.
                        r_sb = opool.tile([P, CH], f16, tag="r")
                        nc.scalar.activation(
                            r_sb[:, :], t_sb[:, :],
                            mybir.ActivationFunctionType.Relu,
                            bias=b2_sb[:, 0:1], scale=1.0 - alpha,
                        )
                        # outT = u + r = leaky(t + b)
                        nc.vector.tensor_tensor(
                            o_full[:, c * CH:(c + 1) * CH], u_sb[:, :],
                            r_sb[:, :], mybir.AluOpType.add,
                        )
                nc.sync.dma_start(outT[b], o_full[:, :])

    _split_multi_waits(nc)
    return nc


def _split_multi_waits(nc):
    """Walrus rejects split-struct instructions with more than one sync wait
    ("Too many sync wait commands" in setupSyncWait<...>). Hoist all but the
    last wait of each multi-wait instruction onto same-engine no-ops inserted
    immediately before it (one wait per no-op)."""
    cnt = 0
    for f in nc.m.functions:
        for blk in f.blocks:
            idx = 0
            while idx < len(blk.instructions):
                inst = blk.instructions[idx]
                si = inst.sync_info
                if (type(inst).__name__ != "InstNoOp" and si is not None
                        and len(si.on_wait) > 1):
                    waits = list(si.on_wait)
                    for w in waits[:-1]:
                        nop = mybir.InstNoOp(name=f"mm_wait_nop_{cnt}",
                                             ins=[], outs=[])
                        cnt += 1
                        nop.engine = inst.engine
                        nop.sync_info = mybir.SyncInfo(on_wait=[w],
                                                       on_update=[])
                        nc.register_instruction(nop)
                        blk.instructions.insert(idx, nop)
                        idx += 1
                    inst.sync_info = mybir.SyncInfo(
                        on_wait=waits[-1:], on_update=list(si.on_update))
                idx += 1
    return cnt


_NC_CACHE = {}


def _get_nc():
    if "nc" not in _NC_CACHE:
        _NC_CACHE["nc"] = build_bass()
    return _NC_CACHE["nc"]


def _prep_in_maps(node_mat, adj_mat, W, b):
    node_mat = np.asarray(node_mat, dtype=np.float32)
    adj_mat = np.asarray(adj_mat, dtype=np.float32)
    wT = np.ascontiguousarray(np.asarray(W, dtype=np.float32).T
                              .astype(np.float16))
    bvec = np.ascontiguousarray(
        np.asarray(b, dtype=np.float32).reshape(P, 1))
    onesW = np.ones((P, P), dtype=np.float16)
    in_maps = []
    for c in range(N_CORES):
        sl = slice(c * BPC, (c + 1) * BPC)
        adjT = np.ascontiguousarray(
            adj_mat[sl].astype(np.float16).transpose(0, 2, 1))
        xs = np.ascontiguousarray(
            node_mat[sl].astype(np.float16)
            .reshape(BPC, N // P, P, FIN).transpose(0, 2, 1, 3))
        in_maps.append({
            "adjT": adjT,
            "x": xs,
            "onesW": onesW,
            "wT": wT,
            "bvec": bvec,
        })
    return in_maps


def kernel(node_mat, adj_mat, W, b):
    nc = _get_nc()
    in_maps = _prep_in_maps(node_mat, adj_mat, W, b)
    res = run_bass_kernel_spmd(nc, in_maps, core_ids=list(range(N_CORES)))
    return np.ascontiguousarray(
        np.concatenate(
            [res.results[c]["outT"] for c in range(N_CORES)], axis=0
        ).swapaxes(1, 2).astype(np.float32)
    )


# revision 8
# speedup vs baseline: 1.6560x; 1.0536x over previous
"""GNN message-passing layer (normalized-adjacency conv + linear + LeakyReLU)
on 8 Trainium2 NeuronCores, pure data parallel over the batch dim.

Computation (per batch b):
    deg      = adj.sum(-1)                     # [N]
    agg      = (adj / deg[:, None]) @ X        # [N, FIN]
    out      = leakyrelu(agg @ W.T + bias)     # [N, FOUT]

Device-side formulation. adj is host-transposed per batch (adjT[k, m] =
adj[m, k]) so the contraction index k sits on SBUF partitions for both matmul
operands, and everything downstream stays transposed ([feature, node] order)
so all PE work streams 512-wide:
    rawT[f, m]   = sum_k X[k, f] * adjT[k, m]    # X tiles as weights
    degbc[:, m]  = sum_k 1 * adjT[k, m]          # ones weights -> deg
                                                 # broadcast to all partitions
    out2T[o, m]  = sum_f WT[f, o] * rawT[f, m]   # W as weights
    t            = out2T * (1/degbc)             # DVE multiply
    outT[o, m]   = Lrelu(t + b)                  # scalar engine, per-partition b
The DRAM output is [B, FOUT, N] fp16; the host swaps the last two axes and
casts to fp32.

Everything DMA'd is fp16 (half the HBM traffic of fp32; adj/X values are
well inside fp16 range and the 2^-11 rounding is far below the accuracy
gate). The deg reduction over the 8 k-tiles is split: 4 pairwise adds on the
DVE (fp16, 2x mode) fold 8 tiles to 4, then a 4-matmul PSUM accumulation
with ones weights folds the rest and broadcasts deg to all 128 partitions.
"""

import numpy as np

import concourse.bass as bass
import concourse.mybir as mybir
import concourse.tile as tile
from concourse.bass_utils import run_bass_kernel_spmd

P = 128

# Problem shape (hardcoded per the harness contract).
B, N, FIN, FOUT = 32, 1024, 128, 128
NEG_SLOPE = 0.01
N_CORES = 8
BPC = B // N_CORES  # batches per core

USE_LRELU = False


def build_bass(nbatch=BPC, n=N, fin=FIN, fout=FOUT, neg_slope=NEG_SLOPE,
               adj_bufs=8, use_lrelu=USE_LRELU):
    f32 = mybir.dt.float32
    f16 = mybir.dt.float16
    alpha = float(neg_slope)
    nc = bass.Bass()

    KT = n // P          # contraction tiles (8)
    CH = min(512, n)     # matmul moving free dim (one fp32 PSUM bank)
    NCH = n // CH        # moving-dim chunks (2)
    KG = KT // 2         # k-tiles per adj DMA chunk (4)

    # All DRAM layouts are host-staged so every DMA is one fully linear
    # run per partition (128 descriptors/transfer instead of 512+): the
    # Sync-engine HWDGE dispatch cost is descriptor-bound.
    adjT = nc.dram_tensor("adjT", [nbatch, 2, P, KG, n], f16,
                          kind="ExternalInput")
    x = nc.dram_tensor("x", [P, nbatch, n // P, fin], f16,
                       kind="ExternalInput")
    w2 = nc.dram_tensor("w2", [P, 2, P], f16, kind="ExternalInput")
    bvec = nc.dram_tensor("bvec", [P, 1], f32, kind="ExternalInput")
    outT = nc.dram_tensor("outT", [nbatch, fout, n], f16, kind="ExternalOutput")

    with tile.TileContext(nc) as tc:
        with (
            tc.tile_pool(name="const", bufs=1) as cpool,
            tc.tile_pool(name="adj", bufs=adj_bufs) as apool,
            tc.tile_pool(name="xt", bufs=1) as xpool,
            tc.tile_pool(name="raw", bufs=2) as rpool,
            tc.tile_pool(name="post", bufs=4) as opool,
            tc.tile_pool(name="psr", bufs=3, space="PSUM") as ps_raw,
            tc.tile_pool(name="psd", bufs=2, space="PSUM") as ps_deg,
            tc.tile_pool(name="pso", bufs=2, space="PSUM") as ps_out,
        ):
            w2_sb = cpool.tile([P, 2, P], f16, tag="w")
            nc.sync.dma_start(w2_sb[:], w2[:, :, :])
            wT_sb = w2_sb[:, 0, :]
            onesW_sb = w2_sb[:, 1, :]
            b_sb = cpool.tile([P, 1], f32, tag="b")
            nc.sync.dma_start(b_sb[:], bvec[:, :])
            if not use_lrelu:
                b2_sb = cpool.tile([P, 1], f32, tag="b2")
                nc.vector.tensor_scalar_mul(b2_sb[:], b_sb[:], 1.0 - alpha)
            # all 4 batches of X in one 1 MiB DMA
            x_sb = xpool.tile([P, nbatch, KT, fin], f16, tag="x")
            nc.sync.dma_start(x_sb[:], x[:, :, :, :])

            # Prefetch ALL adjacency DMAs up-front (8 x 1 MiB): the DMA
            # engines then stream continuously instead of waiting on the
            # per-batch dependency chain. adj_bufs covers every chunk.
            all_chunks = []
            for b in range(nbatch):
                for c2 in range(2):
                    ac = apool.tile([P, KG, n], f16, tag="adj",
                                    name=f"ac{b}_{c2}")
                    nc.sync.dma_start(ac[:], adjT[b, c2])
                    all_chunks.append(ac)

            for b in range(nbatch):
                adj_chunks = all_chunks[2 * b:2 * b + 2]

                def adj_slice(k, c):
                    return adj_chunks[k // KG][:, k % KG, c * CH:(c + 1) * CH]

                # rawT matmuls, one accumulation group per 512-chunk
                ps_chunks = [
                    ps_raw.tile([P, CH], f32, tag="psraw", name=f"psraw{cc}")
                    for cc in range(NCH)
                ]
                for k in range(KT):
                    for c in range(NCH):
                        nc.tensor.matmul(
                            ps_chunks[c][:, :],
                            x_sb[:, b, k, :],
                            adj_slice(k, c),
                            start=(k == 0),
                            stop=(k == KT - 1),
                        )

                # deg: fold 8 k-tiles to 4 with pairwise DVE adds (fp16 2x);
                # the accumulating ones-weights matmuls below fold the rest
                # and broadcast deg to every output partition.
                def aslc(k):
                    return adj_chunks[k // KG][:, k % KG, :]

                pa = []
                for g in range(KT // 2):
                    pt = rpool.tile([P, n], f16, tag=f"pa{g}")
                    nc.vector.tensor_tensor(
                        pt[:, :], aslc(2 * g), aslc(2 * g + 1),
                        mybir.AluOpType.add)
                    pa.append(pt)

                raw_sb = rpool.tile([P, n], f16, tag="raw")
                for c in range(NCH):
                    nc.scalar.copy(raw_sb[:, c * CH:(c + 1) * CH],
                                   ps_chunks[c][:, :])

                o_full = opool.tile([P, n], f16, tag="ofull")
                for c in range(NCH):
                    # deg broadcast to all partitions via ones weights
                    ps_db = ps_deg.tile([P, CH], f32, tag="psdeg")
                    for g in range(KT // 2):
                        nc.tensor.matmul(
                            ps_db[:, :],
                            onesW_sb[:, :],
                            pa[g][:, c * CH:(c + 1) * CH],
                            start=(g == 0),
                            stop=(g == KT // 2 - 1),
                        )
                    # 1/deg on the scalar engine (reciprocal LUT). bass
                    # refuses Reciprocal directly, so emit a Copy and flip
                    # the func.
                    rec_sb = opool.tile([P, CH], f32, tag="rec")
                    _ai = nc.scalar.activation(
                        rec_sb[:, :], ps_db[:, :],
                        mybir.ActivationFunctionType.Copy, bias=0.0, scale=1.0)
                    _ai.ins.func = mybir.ActivationFunctionType.Reciprocal

                    # out2T[o, m] = sum_f WT[f, o] * rawT[f, m]
                    ps_o = ps_out.tile([P, CH], f32, tag="psout")
                    nc.tensor.matmul(
                        ps_o[:, :],
                        wT_sb[:, :],
                        raw_sb[:, c * CH:(c + 1) * CH],
                        start=True,
                        stop=True,
                    )
                    # t = out2T / deg (fp16 out: faster 16-bit DVE modes
                    # downstream; ~2^-11 relative rounding, negligible)
                    t_sb = opool.tile([P, CH], f16, tag="t")
                    nc.vector.tensor_tensor(
                        t_sb[:, :], ps_o[:, :], rec_sb[:, :],
                        mybir.AluOpType.mult,
                    )
                    if use_lrelu:
                        # outT = Lrelu(t + b), negative slope alpha.
                        # (Unused by default: Lrelu lives in a different
                        # ACT table set than Reciprocal, and the per-batch
                        # ACT_TABLE_LOAD thrash costs ~10 us/core.)
                        nc.scalar.activation(
                            o_full[:, c * CH:(c + 1) * CH], t_sb[:, :],
                            mybir.ActivationFunctionType.Lrelu,
                            bias=b_sb[:, 0:1], scale=1.0, alpha=alpha,
                        )
                    else:
                        # u = alpha * (t + b)
                        u_sb = opool.tile([P, CH], f16, tag="u")
                        nc.vector.tensor_scalar(
                            u_sb[:, :], t_sb[:, :], b_sb[:, 0:1], alpha,
                            mybir.AluOpType.add, mybir.AluOpType.mult,
                        )
                        # r = Relu((1-a)*t + (1-a)*b) = (1-a)*Relu(t+b);
                        # Relu is a filler function present in every ACT
                        # table set, so no set switch vs Reciprocal.
                        r_sb = opool.tile([P, CH], f16, tag="r")
                        nc.scalar.activation(
                            r_sb[:, :], t_sb[:, :],
                            mybir.ActivationFunctionType.Relu,
                            bias=b2_sb[:, 0:1], scale=1.0 - alpha,
                        )
                        # outT = u + r = leaky(t + b)
                        nc.vector.tensor_tensor(
                            o_full[:, c * CH:(c + 1) * CH], u_sb[:, :],
                            r_sb[:, :], mybir.AluOpType.add,
                        )
                nc.sync.dma_start(outT[b], o_full[:, :])

    _split_multi_waits(nc)
    return nc


def _split_multi_waits(nc):
    """Walrus rejects split-struct instructions with more than one sync wait
    ("Too many sync wait commands" in setupSyncWait<...>). Hoist all but the
    last wait of each multi-wait instruction onto same-engine no-ops inserted
    immediately before it (one wait per no-op)."""
    cnt = 0
    for f in nc.m.functions:
        for blk in f.blocks:
            idx = 0
            while idx < len(blk.instructions):
                inst = blk.instructions[idx]
                si = inst.sync_info
                if (type(inst).__name__ != "InstNoOp" and si is not None
                        and len(si.on_wait) > 1):
                    waits = list(si.on_wait)
                    for w in waits[:-1]:
                        nop = mybir.InstNoOp(name=f"mm_wait_nop_{cnt}",
                                             ins=[], outs=[])
                        cnt += 1
                        nop.engine = inst.engine
                        nop.sync_info = mybir.SyncInfo(on_wait=[w],
                                                       on_update=[])
                        nc.register_instruction(nop)
                        blk.instructions.insert(idx, nop)
                        idx += 1
                    inst.sync_info = mybir.SyncInfo(
                        on_wait=waits[-1:], on_update=list(si.on_update))
                idx += 1
    return cnt


_NC_CACHE = {}


def _get_nc():
    if "nc" not in _NC_CACHE:
        _NC_CACHE["nc"] = build_bass()
    return _NC_CACHE["nc"]


def _prep_in_maps(node_mat, adj_mat, W, b):
    node_mat = np.asarray(node_mat, dtype=np.float32)
    adj_mat = np.asarray(adj_mat, dtype=np.float32)
    wT = np.asarray(W, dtype=np.float32).T.astype(np.float16)
    w2 = np.ascontiguousarray(
        np.stack([wT, np.ones((P, P), np.float16)], axis=1))
    bvec = np.ascontiguousarray(
        np.asarray(b, dtype=np.float32).reshape(P, 1))
    KG = N // P // 2
    in_maps = []
    for c in range(N_CORES):
        sl = slice(c * BPC, (c + 1) * BPC)
        # adjT_s[b, c2, p, g, m] = adj[b, m, c2*KG*P + g*P + p]
        adjT = np.ascontiguousarray(
            adj_mat[sl].astype(np.float16)
            .reshape(BPC, N, 2, KG, P).transpose(0, 2, 4, 3, 1))
        # xs[p, b, k, f] = node[b, k*P + p, f]
        xs = np.ascontiguousarray(
            node_mat[sl].astype(np.float16)
            .reshape(BPC, N // P, P, FIN).transpose(2, 0, 1, 3))
        in_maps.append({
            "adjT": adjT,
            "x": xs,
            "w2": w2,
            "bvec": bvec,
        })
    return in_maps


def kernel(node_mat, adj_mat, W, b):
    nc = _get_nc()
    in_maps = _prep_in_maps(node_mat, adj_mat, W, b)
    res = run_bass_kernel_spmd(nc, in_maps, core_ids=list(range(N_CORES)))
    return np.ascontiguousarray(
        np.concatenate(
            [res.results[c]["outT"] for c in range(N_CORES)], axis=0
        ).swapaxes(1, 2).astype(np.float32)
    )
